# revision 26
# baseline (speedup 1.0000x reference)
"""Trainium2 Bass kernel for nn_DecoderCell (Tacotron-style decoder cell with
dynamic convolution attention), data-parallel over batch across 8 NeuronCores.

Contract: kernel(**inputs) takes the FULL unsharded inputs (as produced by
setup_inputs()) and returns the FULL output tuple, matching reference():
(out, alpha2, c2, attn_h, attn_c, r1_h, r1_c, r2_h, r2_c).

Precision scheme: PSUM accumulation, softmax, prior, and all element-wise
state math in fp32; matmul operands in fp16.  The per-row softmax is computed
in groups of 8 batch rows so the big h-context streaming for group g overlaps
the attention compute for group g+1.
"""

from contextlib import ExitStack

import numpy as np

import concourse.bass as bass
import concourse.mybir as mybir
import concourse.tile as tile
from concourse import bacc
from concourse.bass import ts
from concourse.bass_utils import run_bass_kernel_spmd
from concourse.masks import make_identity

# ---- problem constants (hardcoded; must match the reference model) ----
B, T, ENC = 512, 512, 512
N_MELS, R = 80, 2
ATT = 256          # ATTN == DEC == PRE_H == PRE_O
HID = 128
C8, K21 = 8, 21    # conv channels / kernel (static == dynamic)
PL = 11            # prior length
NCORES = 8
BC = B // NCORES   # 64 batch rows per core
HALO = T + 2 * (K21 // 2)   # 532
M40 = 40           # fused conv lhsT columns: f at 0:8, g at 32:40
NMR = N_MELS * R   # 160
NG = 8             # softmax/context group size
WAVE = 4           # c2 staging wave

F32 = mybir.dt.float32
F16 = mybir.dt.float16
Act = mybir.ActivationFunctionType

_PROG = None  # built once, reused across kernel() calls


def _build_program():
    nc = bacc.Bacc("TRN2", target_bir_lowering=False, debug=False,
                   num_devices=NCORES)

    def din(name, shape, dtype=F32):
        return nc.dram_tensor(name, shape, dtype, kind="ExternalInput").ap()

    def dout(name, shape):
        return nc.dram_tensor(name, shape, F32, kind="ExternalOutput").ap()

    d = {}
    # ---------- per-core inputs ----------
    d["h_d"] = din("h", [BC, T, ENC], F16)
    d["yT_d"] = din("yT", [N_MELS, BC], F16)
    d["cT_d"] = din("cT", [ENC, BC], F16)
    d["ahT_d"] = din("ahT", [ATT, BC], F16)
    d["acT_d"] = din("acT", [ATT, BC])
    d["r1hT_d"] = din("r1hT", [ATT, BC], F16)
    d["r1cT_d"] = din("r1cT", [ATT, BC])
    d["r2hT_d"] = din("r2hT", [ATT, BC], F16)
    d["r2cT_d"] = din("r2cT", [ATT, BC])
    d["halo_d"] = din("alpha_halo", [BC, HALO])
    d["halo16_d"] = din("alpha_halo16", [BC, HALO], F16)
    d["pbc_d"] = din("P_bc", [BC, PL])

    # ---------- replicated weights (host pre-transposed to [in, out]) -------
    d["fc1_wT_d"] = din("fc1_wT", [N_MELS, ATT], F16)
    d["fc1_b_d"] = din("fc1_b2", [128, 2])
    d["fc2_wT_d"] = din("fc2_wT", [ATT, ATT], F16)
    d["fc2_b_d"] = din("fc2_b2", [128, 2])
    d["wih_d"] = din("attn_wihT", [ENC + ATT, 4 * ATT], F16)
    d["whh_d"] = din("attn_whhT", [ATT, 4 * ATT], F16)
    d["ab_d"] = din("attn_b2", [128, 8])
    d["Ww_d"] = din("W_wT", [ATT, HID], F16)
    d["Wb_d"] = din("W_b2", [128, 1])
    d["Vw_d"] = din("V_wT", [HID, C8 * K21], F16)
    d["twt_d"] = din("TwT", [C8, HID], F16)
    d["cust_d"] = din("CUstatic", [K21, HID])
    d["tb_d"] = din("T_b2", [128, 1])
    d["vbig_d"] = din("v_big", [128, 2 * NG - 1], F16)
    d["lin_d"] = din("lin_wT", [ENC + ATT, ATT], F16)
    d["linb_d"] = din("lin_b2", [128, 2])
    d["r1wih_d"] = din("r1_wihT", [ATT, 4 * ATT], F16)
    d["r1whh_d"] = din("r1_whhT", [ATT, 4 * ATT], F16)
    d["r1b_d"] = din("r1_b2", [128, 8])
    d["r2wih_d"] = din("r2_wihT", [ATT, 4 * ATT], F16)
    d["r2whh_d"] = din("r2_whhT", [ATT, 4 * ATT], F16)
    d["r2b_d"] = din("r2_b2", [128, 8])
    d["projw_d"] = din("proj_wT", [ATT, NMR], F16)

    # ---------- per-core outputs ----------
    d["outT_o"] = dout("outT", [NMR, BC])
    d["alpha2_o"] = dout("alpha2", [BC, T])
    d["c2_o"] = dout("c2", [BC, T])
    d["ahT_o"] = dout("ahT_o", [ATT, BC])
    d["acT_o"] = dout("acT_o", [ATT, BC])
    d["r1hT_o"] = dout("r1hT_o", [ATT, BC])
    d["r1cT_o"] = dout("r1cT_o", [ATT, BC])
    d["r2hT_o"] = dout("r2hT_o", [ATT, BC])
    d["r2cT_o"] = dout("r2cT_o", [ATT, BC])

    with tile.TileContext(nc) as tc:
        with ExitStack() as ctx:
            wp = ctx.enter_context(tc.tile_pool(name="wp", bufs=1))
            hp = ctx.enter_context(tc.tile_pool(name="hp", bufs=8))
            winp = ctx.enter_context(tc.tile_pool(name="winp", bufs=6))
            fgp = ctx.enter_context(tc.tile_pool(name="fgp", bufs=3))
            thp = ctx.enter_context(tc.tile_pool(name="thp", bufs=3))
            gp = ctx.enter_context(tc.tile_pool(name="gp", bufs=10))
            tp = ctx.enter_context(tc.tile_pool(name="tp", bufs=4))
            stp = ctx.enter_context(tc.tile_pool(name="stp", bufs=2))
            dps = ctx.enter_context(
                tc.tile_pool(name="dps", bufs=2, space=bass.MemorySpace.PSUM))
            cups = ctx.enter_context(
                tc.tile_pool(name="cups", bufs=1, space=bass.MemorySpace.PSUM))
            eps = ctx.enter_context(
                tc.tile_pool(name="eps", bufs=1, space=bass.MemorySpace.PSUM))
            egps = ctx.enter_context(
                tc.tile_pool(name="egps", bufs=1, space=bass.MemorySpace.PSUM))
            ctxps = ctx.enter_context(
                tc.tile_pool(name="ctxps", bufs=3,
                             space=bass.MemorySpace.PSUM))
            _kernel_body(nc, tc, d, wp, hp, winp, fgp, thp, gp, tp, stp,
                         dps, cups, eps, egps, ctxps)

    nc.compile()
    return nc


def _kernel_body(nc, tc, d, wp, hp, winp, fgp, thp, gp, tp, stp,
                 dps, cups, eps, egps, ctxps):
    # ---------- load persistent tensors (HWDGE ring; no dtype casts) --------
    def loadT(dram, K, M, tag, dtype=F32, eng=None):
        eng = eng or nc.sync
        if K <= 128:
            t = wp.tile([K, M], dtype, tag=tag)
            eng.dma_start(t[:], dram[:])
            return t
        kc = K // 128
        t = wp.tile([128, kc, M], dtype, tag=tag)
        eng.dma_start(t[:], dram.rearrange("(a p) m -> p a m", p=128))
        return t

    def loadB(dram, n, tag):
        t = wp.tile([128, n], F32, tag=tag)
        nc.sync.dma_start(t[:], dram[:])
        return t

    # order matters: early-phase tensors first so compute can start ASAP
    yT_t = loadT(d["yT_d"], N_MELS, BC, "yT", F16)
    fc1_t = loadT(d["fc1_wT_d"], N_MELS, ATT, "fc1", F16)
    fc1_b = loadB(d["fc1_b_d"], 2, "fc1b")
    fc2_t = loadT(d["fc2_wT_d"], ATT, ATT, "fc2", F16)
    fc2_b = loadB(d["fc2_b_d"], 2, "fc2b")
    cT_t = loadT(d["cT_d"], ENC, BC, "cT", F16)
    ahT_t = loadT(d["ahT_d"], ATT, BC, "ahT", F16)
    acT_t = loadT(d["acT_d"], ATT, BC, "acT")
    wih_t = loadT(d["wih_d"], ENC + ATT, 4 * ATT, "wih", F16)
    whh_t = loadT(d["whh_d"], ATT, 4 * ATT, "whh", F16)
    ab_b = loadB(d["ab_d"], 8, "ab")
    Ww_t = loadT(d["Ww_d"], ATT, HID, "Ww", F16)
    Wb_b = loadB(d["Wb_d"], 1, "Wb")
    Vw_t = loadT(d["Vw_d"], HID, C8 * K21, "Vw", F16)
    twt_t = loadT(d["twt_d"], C8, HID, "twt", F16)
    cust_t = loadT(d["cust_d"], K21, HID, "cust")
    tb_b = loadB(d["tb_d"], 1, "tb")
    vbig_t = loadT(d["vbig_d"], 128, 2 * NG - 1, "vbig", F16)
    halo_t = wp.tile([BC, HALO], F32, tag="halo")
    nc.sync.dma_start(halo_t[:], d["halo_d"][:])
    pbc_t = wp.tile([BC, PL], F32, tag="pbc")
    nc.sync.dma_start(pbc_t[:], d["pbc_d"][:])
    lin_t = loadT(d["lin_d"], ENC + ATT, ATT, "lin", F16, eng=nc.scalar)
    linb_b = loadB(d["linb_d"], 2, "linb")
    r1wih_t = loadT(d["r1wih_d"], ATT, 4 * ATT, "r1wih", F16, eng=nc.scalar)
    r1whh_t = loadT(d["r1whh_d"], ATT, 4 * ATT, "r1whh", F16, eng=nc.scalar)
    r1b_b = loadB(d["r1b_d"], 8, "r1b")
    r1hT_t = loadT(d["r1hT_d"], ATT, BC, "r1hT", F16, eng=nc.scalar)
    r1cT_t = loadT(d["r1cT_d"], ATT, BC, "r1cT")
    r2wih_t = loadT(d["r2wih_d"], ATT, 4 * ATT, "r2wih", F16, eng=nc.scalar)
    r2whh_t = loadT(d["r2whh_d"], ATT, 4 * ATT, "r2whh", F16, eng=nc.scalar)
    r2b_b = loadB(d["r2b_d"], 8, "r2b")
    r2hT_t = loadT(d["r2hT_d"], ATT, BC, "r2hT", F16, eng=nc.scalar)
    r2cT_t = loadT(d["r2cT_d"], ATT, BC, "r2cT")
    projw_t = loadT(d["projw_d"], ATT, NMR, "projw", F16, eng=nc.scalar)

    ident = wp.tile([128, 128], F32, tag="ident")
    make_identity(nc, ident[:])

    # ---------- prenet ----------
    h1 = []
    for mo in range(2):
        ps = dps.tile([128, BC], F32, tag="dps")
        nc.tensor.matmul(ps[:], fc1_t[:, ts(mo, 128)], yT_t[:])
        g = gp.tile([128, BC], F16, tag="h1")
        nc.scalar.activation(g[:], ps[:], Act.Relu, bias=fc1_b[:, mo:mo + 1])
        h1.append(g)
    yp = []
    for mo in range(2):
        ps = dps.tile([128, BC], F32, tag="dps")
        for ki in range(2):
            nc.tensor.matmul(ps[:], fc2_t[:, ki, ts(mo, 128)], h1[ki][:],
                             start=(ki == 0), stop=(ki == 1))
        g = gp.tile([128, BC], F16, tag="yp")
        nc.scalar.activation(g[:], ps[:], Act.Relu, bias=fc2_b[:, mo:mo + 1])
        yp.append(g)

    # ---------- generic LSTM cell ----------
    # x_parts/h_parts are fp16 matmul operands; c_parts/gates/outputs fp32
    def lstm(x_parts, wih_tile, whh_tile, h_parts, c_parts, b_tile, tag):
        gates = []
        for mo in range(8):
            ps = dps.tile([128, BC], F32, tag="dps")
            nk = len(x_parts) + 2
            i = 0
            for ki in range(len(x_parts)):
                nc.tensor.matmul(ps[:], wih_tile[:, ki, ts(mo, 128)],
                                 x_parts[ki], start=(i == 0),
                                 stop=(i == nk - 1))
                i += 1
            for kh in range(2):
                nc.tensor.matmul(ps[:], whh_tile[:, kh, ts(mo, 128)],
                                 h_parts[kh], start=(i == 0),
                                 stop=(i == nk - 1))
                i += 1
            fn = Act.Tanh if mo in (4, 5) else Act.Sigmoid
            g = gp.tile([128, BC], F32, tag=f"g{tag}")
            nc.scalar.activation(g[:], ps[:], fn, bias=b_tile[:, mo:mo + 1])
            gates.append(g)
        h_new, c_new = [], []
        for ch in range(2):
            i_s, f_s = gates[0 + ch], gates[2 + ch]
            g_t, o_s = gates[4 + ch], gates[6 + ch]
            t1 = tp.tile([128, BC], F32, tag="t1")
            nc.vector.tensor_mul(t1[:], f_s[:], c_parts[ch])
            t2 = tp.tile([128, BC], F32, tag="t2")
            nc.vector.tensor_mul(t2[:], i_s[:], g_t[:])
            cn = wp.tile([128, BC], F32, tag=f"cn{tag}{ch}")
            nc.vector.tensor_add(cn[:], t1[:], t2[:])
            t3 = tp.tile([128, BC], F32, tag="t3")
            nc.scalar.activation(t3[:], cn[:], Act.Tanh)
            hn = wp.tile([128, BC], F32, tag=f"hn{tag}{ch}")
            nc.vector.tensor_mul(hn[:], o_s[:], t3[:])
            h_new.append(hn)
            c_new.append(cn)
        return h_new, c_new

    # ---------- attention LSTM ----------
    x_attn = [cT_t[:, ki, :] for ki in range(4)] + [yp[0][:], yp[1][:]]
    ah_new, ac_new = lstm(x_attn, wih_t, whh_t,
                          [ahT_t[:, kh, :] for kh in range(2)],
                          [acT_t[:, kh, :] for kh in range(2)], ab_b, "a")
    ah16 = []
    for chh in range(2):
        nc.gpsimd.dma_start(d["ahT_o"][ts(chh, 128), :], ah_new[chh][:])
        nc.gpsimd.dma_start(d["acT_o"][ts(chh, 128), :], ac_new[chh][:])
        h16 = wp.tile([128, BC], F16, tag=f"ah16{chh}")
        nc.vector.tensor_copy(h16[:], ah_new[chh][:])
        ah16.append(h16)

    # ---------- dynamic filters G ----------
    ps = dps.tile([128, BC], F32, tag="dps")
    for kh in range(2):
        nc.tensor.matmul(ps[:], Ww_t[:, kh, 0:128], ah16[kh][:],
                         start=(kh == 0), stop=(kh == 1))
    th = gp.tile([128, BC], F16, tag="th")
    nc.scalar.activation(th[:], ps[:], Act.Tanh, bias=Wb_b[:, 0:1])

    # G in [c, k, b] layout directly: per k, the 8 channel rows come from
    # V_w columns {c*21+k} (column-strided lhsT slice), so no DRAM round trip
    G_sb = wp.tile([C8, K21, BC], F16, tag="gsb")
    Vw_r = Vw_t[:].rearrange("p (c k) -> p k c", k=K21)
    for k in range(K21):
        gps = dps.tile([C8, BC], F32, tag="dps")
        nc.tensor.matmul(gps[:], Vw_r[:, k, :], th[:])
        nc.vector.tensor_copy(G_sb[:, k, :], gps[:])

    # ---------- prior: p = log(clip(conv(alpha, P), 1e-6)) (DVE, fp32) ------
    pacc = wp.tile([BC, T], F32, tag="pacc")
    nc.vector.tensor_scalar(pacc[:], halo_t[:, 0:T], pbc_t[:, 0:1], None,
                            op0=mybir.AluOpType.mult)
    for k in range(1, PL):
        tmpp = tp.tile([BC, T], F32, tag="ptmp")
        nc.vector.tensor_scalar(tmpp[:], halo_t[:, k:k + T],
                                pbc_t[:, k:k + 1], None,
                                op0=mybir.AluOpType.mult)
        nc.vector.tensor_add(pacc[:], pacc[:], tmpp[:])
    nc.vector.tensor_scalar(pacc[:], pacc[:], 1e-6, None,
                            op0=mybir.AluOpType.max)
    logp = wp.tile([BC, T], F32, tag="logp")
    nc.scalar.activation(logp[:], pacc[:], Act.Ln)
    lgs = []
    for g in range(BC // NG):
        lg = wp.tile([NG, T], F32, tag=f"logpg{g}", name=f"logpg{g}")
        nc.gpsimd.dma_start(lg[:], logp[ts(g, NG), :])
        lgs.append(lg)

    # ---------- grouped DCA + softmax + context (interleaved) --------------
    # Per group of NG batch rows: logits accumulate in a [NG, T] PSUM bank via
    # conv -> e_pre -> tanh -> v-embed; group softmax; transpose the group's
    # alpha2 into fp16 a2T columns; then stream this group's h and reduce.
    # Tile's scheduler overlaps group g's h streaming with group g+1's DCA.
    c2_rows = wp.tile([BC, T], F32, tag="c2rows")
    a2T = [wp.tile([128, BC], F16, tag=f"a2T{i}", name=f"a2T{i}")
           for i in range(4)]
    for g in range(BC // NG):
        e_ps = egps.tile([NG, T], F32, tag="eg")
        for bg in range(NG):
            b = g * NG + bg
            win = winp.tile([K21, T], F16, tag="win")
            src = bass.AP(tensor=d["halo16_d"].tensor, offset=b * HALO,
                          ap=[[1, K21], [1, T]])
            nc.gpsimd.dma_start(win[:], src)

            cu_ps = cups.tile([K21, HID], F32, tag="cu")
            nc.tensor.matmul(cu_ps[:], G_sb[:, :, b], twt_t[:],
                             skip_group_check=True)
            cu_sb = fgp.tile([K21, HID], F16, tag="cusb")
            nc.vector.tensor_add(cu_sb[:], cu_ps[:], cust_t[:])

            e_pre = eps.tile([128, T], F32, tag="eps")
            nc.tensor.matmul(e_pre[:], cu_sb[:], win[:],
                             skip_group_check=True)
            th_sb = thp.tile([128, T], F16, tag="thsb")
            nc.scalar.activation(th_sb[:], e_pre[:], Act.Tanh,
                                 bias=tb_b[:, 0:1])

            nc.tensor.matmul(e_ps[:],
                             vbig_t[:, NG - 1 - bg:2 * NG - 1 - bg],
                             th_sb[:], start=(bg == 0), stop=(bg == NG - 1),
                             skip_group_check=True)

        esb = tp.tile([NG, T], F32, tag="esbg")
        nc.vector.tensor_add(esb[:], e_ps[:], lgs[g][:])
        mx = tp.tile([NG, 1], F32, tag="mxg")
        nc.vector.tensor_reduce(mx[:], esb[:], axis=mybir.AxisListType.X,
                                op=mybir.AluOpType.max)
        nmx = tp.tile([NG, 1], F32, tag="nmxg")
        nc.vector.tensor_scalar(nmx[:], mx[:], -1.0, None,
                                op0=mybir.AluOpType.mult)
        esum = tp.tile([NG, 1], F32, tag="esumg")
        expv = tp.tile([NG, T], F32, tag="expvg")
        nc.scalar.activation(expv[:], esb[:], Act.Exp, bias=nmx[:, 0:1],
                             accum_out=esum[:])
        rinv = tp.tile([NG, 1], F32, tag="rinvg")
        nc.vector.reciprocal(rinv[:], esum[:])
        a2g = fgp.tile([NG, T], F32, tag="a2g")
        nc.vector.tensor_scalar(a2g[:], expv[:], rinv[:, 0:1], None,
                                op0=mybir.AluOpType.mult)
        nc.gpsimd.dma_start(d["alpha2_o"][ts(g, NG), :], a2g[:])

        # transpose the group's alpha2 into fp16 a2T column blocks:
        # a2T[r][tp, b] = alpha2[b, 4*tp + r]
        a2pg = a2g[:].rearrange("b (t four) -> b four t", four=4)
        for r in range(4):
            pst = dps.tile([128, NG], F32, tag="dps")
            nc.tensor.transpose(pst[:], a2pg[:, r, :], ident[0:NG, 0:NG])
            nc.vector.tensor_copy(a2T[r][:, ts(g, NG)], pst[:])

        # context reduction for this group's rows: one 512KB DMA per row
        # (partition tp holds t = 4*tp + r), rotating across all DMA rings
        for w in range(NG // WAVE):
            stage = stp.tile([1, WAVE, T], F32, tag="c2stage")
            for bw in range(WAVE):
                b = g * NG + w * WAVE + bw
                cps = ctxps.tile([1, ENC], F32, tag="ctx")
                ht = hp.tile([128, 4 * ENC], F16, tag="ht")
                hsrc = bass.AP(tensor=d["h_d"].tensor, offset=b * T * ENC,
                               ap=[[4 * ENC, 128], [1, 4 * ENC]])
                eng = (nc.sync, nc.scalar, nc.gpsimd)[b % 3]
                eng.dma_start(ht[:], hsrc)
                for r in range(4):
                    nc.tensor.matmul(
                        cps[:], a2T[r][:, b:b + 1],
                        ht[:, r * ENC:(r + 1) * ENC],
                        start=(r == 0), stop=(r == 3),
                        skip_group_check=True)
                nc.vector.tensor_copy(stage[0:1, bw, :], cps[:])
            nc.gpsimd.dma_start(c2_rows[ts(g * (NG // WAVE) + w, WAVE), :],
                                stage[:])
    nc.gpsimd.dma_start(d["c2_o"][:], c2_rows[:])

    c2T = []
    for dc in range(4):
        pst = dps.tile([128, BC], F32, tag="dps")
        nc.tensor.transpose(pst[:], c2_rows[:, ts(dc, 128)],
                            ident[0:BC, 0:BC])
        c = wp.tile([128, BC], F16, tag=f"c2T{dc}")
        nc.vector.tensor_copy(c[:], pst[:])
        c2T.append(c)

    # ---------- lin + decoder LSTMs + proj ----------
    x_lin = [c2T[i][:] for i in range(4)] + [ah16[0][:], ah16[1][:]]
    xT, xT16 = [], []
    for mo in range(2):
        ps2 = dps.tile([128, BC], F32, tag="dps")
        for ki in range(6):
            nc.tensor.matmul(ps2[:], lin_t[:, ki, ts(mo, 128)], x_lin[ki],
                             start=(ki == 0), stop=(ki == 5))
        xt_ = wp.tile([128, BC], F32, tag=f"xT{mo}")
        nc.scalar.activation(xt_[:], ps2[:], Act.Identity,
                             bias=linb_b[:, mo:mo + 1])
        xT.append(xt_)
        x16 = wp.tile([128, BC], F16, tag=f"xT16{mo}")
        nc.vector.tensor_copy(x16[:], xt_[:])
        xT16.append(x16)

    r1h, r1c = lstm([xT16[0][:], xT16[1][:]], r1wih_t, r1whh_t,
                    [r1hT_t[:, kh, :] for kh in range(2)],
                    [r1cT_t[:, kh, :] for kh in range(2)], r1b_b, "r1")
    x2, x2_16 = [], []
    for chh in range(2):
        nc.gpsimd.dma_start(d["r1hT_o"][ts(chh, 128), :], r1h[chh][:])
        nc.gpsimd.dma_start(d["r1cT_o"][ts(chh, 128), :], r1c[chh][:])
        xx = wp.tile([128, BC], F32, tag=f"x2{chh}")
        nc.vector.tensor_add(xx[:], xT[chh][:], r1h[chh][:])
        x2.append(xx)
        xx16 = wp.tile([128, BC], F16, tag=f"x216{chh}")
        nc.vector.tensor_copy(xx16[:], xx[:])
        x2_16.append(xx16)

    r2h, r2c = lstm([x2_16[0][:], x2_16[1][:]], r2wih_t, r2whh_t,
                    [r2hT_t[:, kh, :] for kh in range(2)],
                    [r2cT_t[:, kh, :] for kh in range(2)], r2b_b, "r2")
    x3 = []
    for chh in range(2):
        nc.gpsimd.dma_start(d["r2hT_o"][ts(chh, 128), :], r2h[chh][:])
        nc.gpsimd.dma_start(d["r2cT_o"][ts(chh, 128), :], r2c[chh][:])
        xx = wp.tile([128, BC], F16, tag=f"x3{chh}")
        nc.vector.tensor_add(xx[:], x2[chh][:], r2h[chh][:])
        x3.append(xx)

    ps3 = dps.tile([128, BC], F32, tag="dps")
    for ki in range(2):
        nc.tensor.matmul(ps3[:], projw_t[:, ki, 0:128], x3[ki][:],
                         start=(ki == 0), stop=(ki == 1))
    o0 = tp.tile([128, BC], F32, tag="o0")
    nc.vector.tensor_copy(o0[:], ps3[:])
    nc.gpsimd.dma_start(d["outT_o"][0:128, :], o0[:])
    ps4 = dps.tile([32, BC], F32, tag="dps")
    for ki in range(2):
        nc.tensor.matmul(ps4[:], projw_t[:, ki, 128:160], x3[ki][:],
                         start=(ki == 0), stop=(ki == 1))
    o1 = tp.tile([32, BC], F32, tag="o1")
    nc.vector.tensor_copy(o1[:], ps4[:])
    nc.gpsimd.dma_start(d["outT_o"][128:160, :], o1[:])


def _host_prepare(inputs):
    """Build the 8 per-core input maps from the full inputs."""
    f = np.float32
    f16 = np.float16
    npa = {k: np.asarray(v, dtype=f) for k, v in inputs.items()}

    def TT(a, dt=f):
        return np.ascontiguousarray(a.T.astype(dt))

    def b2(bias, n):
        return np.ascontiguousarray(np.asarray(bias, f).reshape(n, 128).T)

    wk = {
        "fc1_wT": TT(npa["fc1_w"], f16),
        "fc1_b2": b2(npa["fc1_b"], 2),
        "fc2_wT": TT(npa["fc2_w"], f16),
        "fc2_b2": b2(npa["fc2_b"], 2),
        "attn_wihT": TT(npa["attn_wih"], f16),
        "attn_whhT": TT(npa["attn_whh"], f16),
        "attn_b2": b2(npa["attn_bih"] + npa["attn_bhh"], 8),
        "W_wT": TT(npa["W_w"], f16),
        "W_b2": b2(npa["W_b"], 1),
        "V_wT": TT(npa["V_w"], f16),
        "T_b2": b2(npa["T_b"], 1),
        "lin_wT": TT(npa["lin_w"], f16),
        "lin_b2": b2(npa["lin_b"], 2),
        "r1_wihT": TT(npa["r1_wih"], f16),
        "r1_whhT": TT(npa["r1_whh"], f16),
        "r1_b2": b2(npa["r1_bih"] + npa["r1_bhh"], 8),
        "r2_wihT": TT(npa["r2_wih"], f16),
        "r2_whhT": TT(npa["r2_whh"], f16),
        "r2_b2": b2(npa["r2_bih"] + npa["r2_bhh"], 8),
        "proj_wT": TT(npa["proj_w"], f16),
    }
    wk["TwT"] = np.ascontiguousarray(npa["T_w"].T.astype(f16))
    wk["CUstatic"] = np.ascontiguousarray(
        npa["F_w"][:, 0, :].T @ npa["U_w"].T)
    vbig = np.zeros((128, 2 * NG - 1), f16)
    vbig[:, NG - 1] = npa["v_w"][0].astype(f16)
    wk["v_big"] = vbig

    pbc = np.ascontiguousarray(
        np.broadcast_to(npa["P"][None, :], (BC, PL)).astype(f))

    in_maps = []
    for c in range(NCORES):
        sl = slice(c * BC, (c + 1) * BC)
        m = dict(wk)
        m["h"] = np.ascontiguousarray(npa["h"][sl]).astype(f16)
        m["yT"] = TT(npa["y"][sl], f16)
        m["cT"] = TT(npa["c"][sl], f16)
        m["ahT"] = TT(npa["attn_h0"][sl], f16)
        m["acT"] = TT(npa["attn_c0"][sl])
        m["r1hT"] = TT(npa["rnn1_h0"][sl], f16)
        m["r1cT"] = TT(npa["rnn1_c0"][sl])
        m["r2hT"] = TT(npa["rnn2_h0"][sl], f16)
        m["r2cT"] = TT(npa["rnn2_c0"][sl])
        halo = np.ascontiguousarray(
            np.pad(npa["alpha"][sl], ((0, 0), (K21 // 2, K21 // 2))))
        m["alpha_halo"] = halo
        m["alpha_halo16"] = halo.astype(f16)
        m["P_bc"] = pbc
        in_maps.append(m)
    return in_maps


def _assemble(results):
    outs, a2s, c2s = [], [], []
    ahs, acs, r1hs, r1cs, r2hs, r2cs = [], [], [], [], [], []
    for r in results:
        outs.append(r["outT"].T.reshape(BC, N_MELS, R))
        a2s.append(r["alpha2"])
        c2s.append(r["c2"])
        ahs.append(r["ahT_o"].T)
        acs.append(r["acT_o"].T)
        r1hs.append(r["r1hT_o"].T)
        r1cs.append(r["r1cT_o"].T)
        r2hs.append(r["r2hT_o"].T)
        r2cs.append(r["r2cT_o"].T)

    def cat(xs):
        return np.ascontiguousarray(np.concatenate(xs, axis=0))

    return (cat(outs), cat(a2s), cat(c2s), cat(ahs), cat(acs),
            cat(r1hs), cat(r1cs), cat(r2hs), cat(r2cs))


def get_program():
    global _PROG
    if _PROG is None:
        _PROG = _build_program()
    return _PROG


def kernel(**inputs):
    nc = get_program()
    in_maps = _host_prepare(inputs)
    res = run_bass_kernel_spmd(nc, in_maps, list(range(NCORES)))
    return _assemble(res.results)


if __name__ == "__main__":
    get_program()
    print("program built OK")


# revision 27
# speedup vs baseline: 1.2428x; 1.2428x over previous
"""Trainium2 Bass kernel for nn_DecoderCell (Tacotron-style decoder cell with
dynamic convolution attention), data-parallel over batch across 8 NeuronCores.

Contract: kernel(**inputs) takes the FULL unsharded inputs (as produced by
setup_inputs()) and returns the FULL output tuple, matching reference():
(out, alpha2, c2, attn_h, attn_c, r1_h, r1_c, r2_h, r2_c).

Precision scheme: PSUM accumulation, softmax, prior, and all element-wise
state math in fp32; matmul operands in fp16.  The per-row softmax is computed
in groups of 8 batch rows so the big h-context streaming for group g overlaps
the attention compute for group g+1.
"""

from contextlib import ExitStack

import numpy as np

import concourse.bass as bass
import concourse.mybir as mybir
import concourse.tile as tile
from concourse import bacc
from concourse.bass import ts
from concourse.bass_utils import run_bass_kernel_spmd
from concourse.masks import make_identity

# ---- problem constants (hardcoded; must match the reference model) ----
B, T, ENC = 512, 512, 512
N_MELS, R = 80, 2
ATT = 256          # ATTN == DEC == PRE_H == PRE_O
HID = 128
C8, K21 = 8, 21    # conv channels / kernel (static == dynamic)
PL = 11            # prior length
NCORES = 8
BC = B // NCORES   # 64 batch rows per core
HALO = T + 2 * (K21 // 2)   # 532
M40 = 40           # fused conv lhsT columns: f at 0:8, g at 32:40
NMR = N_MELS * R   # 160
NG = 8             # softmax/context group size
WAVE = 4           # c2 staging wave

F32 = mybir.dt.float32
F16 = mybir.dt.float16
Act = mybir.ActivationFunctionType

_PROG = None  # built once, reused across kernel() calls


def _build_program():
    nc = bacc.Bacc("TRN2", target_bir_lowering=False, debug=False,
                   num_devices=NCORES)

    def din(name, shape, dtype=F32):
        return nc.dram_tensor(name, shape, dtype, kind="ExternalInput").ap()

    def dout(name, shape):
        return nc.dram_tensor(name, shape, F32, kind="ExternalOutput").ap()

    d = {}
    # ---------- per-core inputs ----------
    d["h_d"] = din("h", [BC, T, ENC], F16)
    d["yT_d"] = din("yT", [N_MELS, BC], F16)
    d["cT_d"] = din("cT", [ENC, BC], F16)
    d["ahT_d"] = din("ahT", [ATT, BC], F16)
    d["acT_d"] = din("acT", [ATT, BC])
    d["r1hT_d"] = din("r1hT", [ATT, BC], F16)
    d["r1cT_d"] = din("r1cT", [ATT, BC])
    d["r2hT_d"] = din("r2hT", [ATT, BC], F16)
    d["r2cT_d"] = din("r2cT", [ATT, BC])
    d["halo_d"] = din("alpha_halo", [BC, HALO])
    d["halo16_d"] = din("alpha_halo16", [BC, HALO], F16)
    d["pbc_d"] = din("P_bc", [BC, PL])

    # ---------- replicated weights (host pre-transposed to [in, out]) -------
    d["fc1_wT_d"] = din("fc1_wT", [N_MELS, ATT], F16)
    d["fc1_b_d"] = din("fc1_b2", [128, 2])
    d["fc2_wT_d"] = din("fc2_wT", [ATT, ATT], F16)
    d["fc2_b_d"] = din("fc2_b2", [128, 2])
    d["wih_d"] = din("attn_wihT", [ENC + ATT, 4 * ATT], F16)
    d["whh_d"] = din("attn_whhT", [ATT, 4 * ATT], F16)
    d["ab_d"] = din("attn_b2", [128, 8])
    d["Ww_d"] = din("W_wT", [ATT, HID], F16)
    d["Wb_d"] = din("W_b2", [128, 1])
    d["Vw_d"] = din("V_wT", [HID, C8 * K21], F16)
    d["twt_d"] = din("TwT", [C8, HID], F16)
    d["cust_d"] = din("CUstatic", [K21, HID])
    d["tb_d"] = din("T_b2", [128, 1])
    d["vbig_d"] = din("v_big", [128, 2 * NG - 1], F16)
    d["lin_d"] = din("lin_wT", [ENC + ATT, ATT], F16)
    d["linb_d"] = din("lin_b2", [128, 2])
    d["r1wih_d"] = din("r1_wihT", [ATT, 4 * ATT], F16)
    d["r1whh_d"] = din("r1_whhT", [ATT, 4 * ATT], F16)
    d["r1b_d"] = din("r1_b2", [128, 8])
    d["r2wih_d"] = din("r2_wihT", [ATT, 4 * ATT], F16)
    d["r2whh_d"] = din("r2_whhT", [ATT, 4 * ATT], F16)
    d["r2b_d"] = din("r2_b2", [128, 8])
    d["projw_d"] = din("proj_wT", [ATT, NMR], F16)

    # ---------- per-core outputs ----------
    d["outT_o"] = dout("outT", [NMR, BC])
    d["alpha2_o"] = dout("alpha2", [BC, T])
    d["c2_o"] = dout("c2", [BC, T])
    d["ahT_o"] = dout("ahT_o", [ATT, BC])
    d["acT_o"] = dout("acT_o", [ATT, BC])
    d["r1hT_o"] = dout("r1hT_o", [ATT, BC])
    d["r1cT_o"] = dout("r1cT_o", [ATT, BC])
    d["r2hT_o"] = dout("r2hT_o", [ATT, BC])
    d["r2cT_o"] = dout("r2cT_o", [ATT, BC])

    with tile.TileContext(nc) as tc:
        with ExitStack() as ctx:
            wp = ctx.enter_context(tc.tile_pool(name="wp", bufs=1))
            hp = ctx.enter_context(tc.tile_pool(name="hp", bufs=8))
            winp = ctx.enter_context(tc.tile_pool(name="winp", bufs=6))
            fgp = ctx.enter_context(tc.tile_pool(name="fgp", bufs=3))
            thp = ctx.enter_context(tc.tile_pool(name="thp", bufs=3))
            gp = ctx.enter_context(tc.tile_pool(name="gp", bufs=10))
            tp = ctx.enter_context(tc.tile_pool(name="tp", bufs=4))
            stp = ctx.enter_context(tc.tile_pool(name="stp", bufs=2))
            dps = ctx.enter_context(
                tc.tile_pool(name="dps", bufs=2, space=bass.MemorySpace.PSUM))
            cups = ctx.enter_context(
                tc.tile_pool(name="cups", bufs=1, space=bass.MemorySpace.PSUM))
            eps = ctx.enter_context(
                tc.tile_pool(name="eps", bufs=2, space=bass.MemorySpace.PSUM))
            egps = ctx.enter_context(
                tc.tile_pool(name="egps", bufs=1, space=bass.MemorySpace.PSUM))
            ctxps = ctx.enter_context(
                tc.tile_pool(name="ctxps", bufs=2,
                             space=bass.MemorySpace.PSUM))
            _kernel_body(nc, tc, d, wp, hp, winp, fgp, thp, gp, tp, stp,
                         dps, cups, eps, egps, ctxps)

    nc.compile()
    return nc


def _kernel_body(nc, tc, d, wp, hp, winp, fgp, thp, gp, tp, stp,
                 dps, cups, eps, egps, ctxps):
    # ---------- load persistent tensors (HWDGE ring; no dtype casts) --------
    def loadT(dram, K, M, tag, dtype=F32, eng=None):
        eng = eng or nc.sync
        if K <= 128:
            t = wp.tile([K, M], dtype, tag=tag)
            eng.dma_start(t[:], dram[:])
            return t
        kc = K // 128
        t = wp.tile([128, kc, M], dtype, tag=tag)
        eng.dma_start(t[:], dram.rearrange("(a p) m -> p a m", p=128))
        return t

    def loadB(dram, n, tag):
        t = wp.tile([128, n], F32, tag=tag)
        nc.sync.dma_start(t[:], dram[:])
        return t

    # order matters: early-phase tensors first so compute can start ASAP
    yT_t = loadT(d["yT_d"], N_MELS, BC, "yT", F16)
    fc1_t = loadT(d["fc1_wT_d"], N_MELS, ATT, "fc1", F16)
    fc1_b = loadB(d["fc1_b_d"], 2, "fc1b")
    fc2_t = loadT(d["fc2_wT_d"], ATT, ATT, "fc2", F16)
    fc2_b = loadB(d["fc2_b_d"], 2, "fc2b")
    cT_t = loadT(d["cT_d"], ENC, BC, "cT", F16)
    ahT_t = loadT(d["ahT_d"], ATT, BC, "ahT", F16)
    acT_t = loadT(d["acT_d"], ATT, BC, "acT")
    wih_t = loadT(d["wih_d"], ENC + ATT, 4 * ATT, "wih", F16)
    whh_t = loadT(d["whh_d"], ATT, 4 * ATT, "whh", F16)
    ab_b = loadB(d["ab_d"], 8, "ab")
    Ww_t = loadT(d["Ww_d"], ATT, HID, "Ww", F16)
    Wb_b = loadB(d["Wb_d"], 1, "Wb")
    Vw_t = loadT(d["Vw_d"], HID, C8 * K21, "Vw", F16)
    twt_t = loadT(d["twt_d"], C8, HID, "twt", F16)
    cust_t = loadT(d["cust_d"], K21, HID, "cust")
    tb_b = loadB(d["tb_d"], 1, "tb")
    vbig_t = loadT(d["vbig_d"], 128, 2 * NG - 1, "vbig", F16)
    halo_t = wp.tile([BC, HALO], F32, tag="halo")
    nc.sync.dma_start(halo_t[:], d["halo_d"][:])
    pbc_t = wp.tile([BC, PL], F32, tag="pbc")
    nc.sync.dma_start(pbc_t[:], d["pbc_d"][:])
    lin_t = loadT(d["lin_d"], ENC + ATT, ATT, "lin", F16, eng=nc.scalar)
    linb_b = loadB(d["linb_d"], 2, "linb")
    r1wih_t = loadT(d["r1wih_d"], ATT, 4 * ATT, "r1wih", F16, eng=nc.scalar)
    r1whh_t = loadT(d["r1whh_d"], ATT, 4 * ATT, "r1whh", F16, eng=nc.scalar)
    r1b_b = loadB(d["r1b_d"], 8, "r1b")
    r1hT_t = loadT(d["r1hT_d"], ATT, BC, "r1hT", F16, eng=nc.scalar)
    r1cT_t = loadT(d["r1cT_d"], ATT, BC, "r1cT")
    r2wih_t = loadT(d["r2wih_d"], ATT, 4 * ATT, "r2wih", F16, eng=nc.scalar)
    r2whh_t = loadT(d["r2whh_d"], ATT, 4 * ATT, "r2whh", F16, eng=nc.scalar)
    r2b_b = loadB(d["r2b_d"], 8, "r2b")
    r2hT_t = loadT(d["r2hT_d"], ATT, BC, "r2hT", F16, eng=nc.scalar)
    r2cT_t = loadT(d["r2cT_d"], ATT, BC, "r2cT")
    projw_t = loadT(d["projw_d"], ATT, NMR, "projw", F16, eng=nc.scalar)

    ident = wp.tile([128, 128], F32, tag="ident")
    make_identity(nc, ident[:])

    # ---------- prenet ----------
    h1 = []
    for mo in range(2):
        ps = dps.tile([128, BC], F32, tag="dps")
        nc.tensor.matmul(ps[:], fc1_t[:, ts(mo, 128)], yT_t[:])
        g = gp.tile([128, BC], F16, tag="h1")
        nc.scalar.activation(g[:], ps[:], Act.Relu, bias=fc1_b[:, mo:mo + 1])
        h1.append(g)
    yp = []
    for mo in range(2):
        ps = dps.tile([128, BC], F32, tag="dps")
        for ki in range(2):
            nc.tensor.matmul(ps[:], fc2_t[:, ki, ts(mo, 128)], h1[ki][:],
                             start=(ki == 0), stop=(ki == 1))
        g = gp.tile([128, BC], F16, tag="yp")
        nc.scalar.activation(g[:], ps[:], Act.Relu, bias=fc2_b[:, mo:mo + 1])
        yp.append(g)

    # ---------- generic LSTM cell ----------
    # x_parts/h_parts are fp16 matmul operands; c_parts/gates/outputs fp32
    def lstm(x_parts, wih_tile, whh_tile, h_parts, c_parts, b_tile, tag):
        gates = []
        for mo in range(8):
            ps = dps.tile([128, BC], F32, tag="dps")
            nk = len(x_parts) + 2
            i = 0
            for ki in range(len(x_parts)):
                nc.tensor.matmul(ps[:], wih_tile[:, ki, ts(mo, 128)],
                                 x_parts[ki], start=(i == 0),
                                 stop=(i == nk - 1))
                i += 1
            for kh in range(2):
                nc.tensor.matmul(ps[:], whh_tile[:, kh, ts(mo, 128)],
                                 h_parts[kh], start=(i == 0),
                                 stop=(i == nk - 1))
                i += 1
            fn = Act.Tanh if mo in (4, 5) else Act.Sigmoid
            g = gp.tile([128, BC], F32, tag=f"g{tag}")
            nc.scalar.activation(g[:], ps[:], fn, bias=b_tile[:, mo:mo + 1])
            gates.append(g)
        h_new, c_new = [], []
        for ch in range(2):
            i_s, f_s = gates[0 + ch], gates[2 + ch]
            g_t, o_s = gates[4 + ch], gates[6 + ch]
            t1 = tp.tile([128, BC], F32, tag="t1")
            nc.vector.tensor_mul(t1[:], f_s[:], c_parts[ch])
            t2 = tp.tile([128, BC], F32, tag="t2")
            nc.vector.tensor_mul(t2[:], i_s[:], g_t[:])
            cn = wp.tile([128, BC], F32, tag=f"cn{tag}{ch}")
            nc.vector.tensor_add(cn[:], t1[:], t2[:])
            t3 = tp.tile([128, BC], F32, tag="t3")
            nc.scalar.activation(t3[:], cn[:], Act.Tanh)
            hn = wp.tile([128, BC], F32, tag=f"hn{tag}{ch}")
            nc.vector.tensor_mul(hn[:], o_s[:], t3[:])
            h_new.append(hn)
            c_new.append(cn)
        return h_new, c_new

    # ---------- attention LSTM ----------
    x_attn = [cT_t[:, ki, :] for ki in range(4)] + [yp[0][:], yp[1][:]]
    ah_new, ac_new = lstm(x_attn, wih_t, whh_t,
                          [ahT_t[:, kh, :] for kh in range(2)],
                          [acT_t[:, kh, :] for kh in range(2)], ab_b, "a")
    ah16 = []
    for chh in range(2):
        nc.gpsimd.dma_start(d["ahT_o"][ts(chh, 128), :], ah_new[chh][:])
        nc.gpsimd.dma_start(d["acT_o"][ts(chh, 128), :], ac_new[chh][:])
        h16 = wp.tile([128, BC], F16, tag=f"ah16{chh}")
        nc.vector.tensor_copy(h16[:], ah_new[chh][:])
        ah16.append(h16)

    # ---------- dynamic filters G ----------
    ps = dps.tile([128, BC], F32, tag="dps")
    for kh in range(2):
        nc.tensor.matmul(ps[:], Ww_t[:, kh, 0:128], ah16[kh][:],
                         start=(kh == 0), stop=(kh == 1))
    th = gp.tile([128, BC], F16, tag="th")
    nc.scalar.activation(th[:], ps[:], Act.Tanh, bias=Wb_b[:, 0:1])

    # G in [c, k, b] layout directly: per k, the 8 channel rows come from
    # V_w columns {c*21+k} (column-strided lhsT slice), so no DRAM round trip
    G_sb = wp.tile([C8, K21, BC], F16, tag="gsb")
    Vw_r = Vw_t[:].rearrange("p (c k) -> p k c", k=K21)
    for k in range(K21):
        gps = dps.tile([C8, BC], F32, tag="dps")
        nc.tensor.matmul(gps[:], Vw_r[:, k, :], th[:])
        nc.vector.tensor_copy(G_sb[:, k, :], gps[:])

    # ---------- prior: p = log(clip(conv(alpha, P), 1e-6)) (DVE, fp32) ------
    pacc = wp.tile([BC, T], F32, tag="pacc")
    nc.vector.tensor_scalar(pacc[:], halo_t[:, 0:T], pbc_t[:, 0:1], None,
                            op0=mybir.AluOpType.mult)
    for k in range(1, PL):
        tmpp = tp.tile([BC, T], F32, tag="ptmp")
        nc.vector.tensor_scalar(tmpp[:], halo_t[:, k:k + T],
                                pbc_t[:, k:k + 1], None,
                                op0=mybir.AluOpType.mult)
        nc.vector.tensor_add(pacc[:], pacc[:], tmpp[:])
    nc.vector.tensor_scalar(pacc[:], pacc[:], 1e-6, None,
                            op0=mybir.AluOpType.max)
    logp = wp.tile([BC, T], F32, tag="logp")
    nc.scalar.activation(logp[:], pacc[:], Act.Ln)
    lgs = []
    for g in range(BC // NG):
        lg = wp.tile([NG, T], F32, tag=f"logpg{g}", name=f"logpg{g}")
        nc.gpsimd.dma_start(lg[:], logp[ts(g, NG), :])
        lgs.append(lg)

    # ---------- grouped DCA + softmax + context (interleaved) --------------
    # Per group of NG batch rows: logits accumulate in a [NG, T] PSUM bank via
    # conv -> e_pre -> tanh -> v-embed; group softmax; transpose the group's
    # alpha2 into fp16 a2T columns; then stream this group's h and reduce.
    # Tile's scheduler overlaps group g's h streaming with group g+1's DCA.
    c2_rows = wp.tile([BC, T], F32, tag="c2rows")
    a2T = [wp.tile([128, BC], F16, tag=f"a2T{i}", name=f"a2T{i}")
           for i in range(4)]
    for g in range(BC // NG):
        e_ps = egps.tile([NG, T], F32, tag="eg")
        for bg in range(NG):
            b = g * NG + bg
            win = winp.tile([K21, T], F16, tag="win")
            src = bass.AP(tensor=d["halo16_d"].tensor, offset=b * HALO,
                          ap=[[1, K21], [1, T]])
            nc.gpsimd.dma_start(win[:], src)

            cu_ps = cups.tile([K21, HID], F32, tag="cu")
            nc.tensor.matmul(cu_ps[:], G_sb[:, :, b], twt_t[:],
                             skip_group_check=True)
            cu_sb = fgp.tile([K21, HID], F16, tag="cusb")
            nc.vector.tensor_add(cu_sb[:], cu_ps[:], cust_t[:])

            e_pre = eps.tile([128, T], F32, tag="eps")
            nc.tensor.matmul(e_pre[:], cu_sb[:], win[:],
                             skip_group_check=True)
            th_sb = thp.tile([128, T], F16, tag="thsb")
            nc.scalar.activation(th_sb[:], e_pre[:], Act.Tanh,
                                 bias=tb_b[:, 0:1])

            nc.tensor.matmul(e_ps[:],
                             vbig_t[:, NG - 1 - bg:2 * NG - 1 - bg],
                             th_sb[:], start=(bg == 0), stop=(bg == NG - 1),
                             skip_group_check=True)

        esb = tp.tile([NG, T], F32, tag="esbg")
        nc.vector.tensor_add(esb[:], e_ps[:], lgs[g][:])
        mx = tp.tile([NG, 1], F32, tag="mxg")
        nc.vector.tensor_reduce(mx[:], esb[:], axis=mybir.AxisListType.X,
                                op=mybir.AluOpType.max)
        nmx = tp.tile([NG, 1], F32, tag="nmxg")
        nc.vector.tensor_scalar(nmx[:], mx[:], -1.0, None,
                                op0=mybir.AluOpType.mult)
        esum = tp.tile([NG, 1], F32, tag="esumg")
        expv = tp.tile([NG, T], F32, tag="expvg")
        nc.scalar.activation(expv[:], esb[:], Act.Exp, bias=nmx[:, 0:1],
                             accum_out=esum[:])
        rinv = tp.tile([NG, 1], F32, tag="rinvg")
        nc.vector.reciprocal(rinv[:], esum[:])
        a2g = fgp.tile([NG, T], F32, tag="a2g")
        nc.vector.tensor_scalar(a2g[:], expv[:], rinv[:, 0:1], None,
                                op0=mybir.AluOpType.mult)
        nc.gpsimd.dma_start(d["alpha2_o"][ts(g, NG), :], a2g[:])

        # transpose the group's alpha2 into fp16 a2T column blocks:
        # a2T[r][tp, b] = alpha2[b, 4*tp + r]
        a2pg = a2g[:].rearrange("b (t four) -> b four t", four=4)
        for r in range(4):
            pst = dps.tile([128, NG], F32, tag="dps")
            nc.tensor.transpose(pst[:], a2pg[:, r, :], ident[0:NG, 0:NG])
            nc.vector.tensor_copy(a2T[r][:, ts(g, NG)], pst[:])

        # context reduction for this group's rows: one 512KB DMA per row
        # (partition tp holds t = 4*tp + r), rotating across all DMA rings
        for w in range(NG // WAVE):
            stage = stp.tile([1, WAVE, T], F32, tag="c2stage")
            for bw in range(WAVE):
                b = g * NG + w * WAVE + bw
                cps = ctxps.tile([1, ENC], F32, tag="ctx")
                ht = hp.tile([128, 4 * ENC], F16, tag="ht")
                hsrc = bass.AP(tensor=d["h_d"].tensor, offset=b * T * ENC,
                               ap=[[4 * ENC, 128], [1, 4 * ENC]])
                eng = (nc.sync, nc.scalar, nc.gpsimd)[b % 3]
                eng.dma_start(ht[:], hsrc)
                for r in range(4):
                    nc.tensor.matmul(
                        cps[:], a2T[r][:, b:b + 1],
                        ht[:, r * ENC:(r + 1) * ENC],
                        start=(r == 0), stop=(r == 3),
                        skip_group_check=True)
                nc.vector.tensor_copy(stage[0:1, bw, :], cps[:])
            nc.gpsimd.dma_start(c2_rows[ts(g * (NG // WAVE) + w, WAVE), :],
                                stage[:])
    nc.gpsimd.dma_start(d["c2_o"][:], c2_rows[:])

    c2T = []
    for dc in range(4):
        pst = dps.tile([128, BC], F32, tag="dps")
        nc.tensor.transpose(pst[:], c2_rows[:, ts(dc, 128)],
                            ident[0:BC, 0:BC])
        c = wp.tile([128, BC], F16, tag=f"c2T{dc}")
        nc.vector.tensor_copy(c[:], pst[:])
        c2T.append(c)

    # ---------- lin + decoder LSTMs + proj ----------
    x_lin = [c2T[i][:] for i in range(4)] + [ah16[0][:], ah16[1][:]]
    xT, xT16 = [], []
    for mo in range(2):
        ps2 = dps.tile([128, BC], F32, tag="dps")
        for ki in range(6):
            nc.tensor.matmul(ps2[:], lin_t[:, ki, ts(mo, 128)], x_lin[ki],
                             start=(ki == 0), stop=(ki == 5))
        xt_ = wp.tile([128, BC], F32, tag=f"xT{mo}")
        nc.scalar.activation(xt_[:], ps2[:], Act.Identity,
                             bias=linb_b[:, mo:mo + 1])
        xT.append(xt_)
        x16 = wp.tile([128, BC], F16, tag=f"xT16{mo}")
        nc.vector.tensor_copy(x16[:], xt_[:])
        xT16.append(x16)

    r1h, r1c = lstm([xT16[0][:], xT16[1][:]], r1wih_t, r1whh_t,
                    [r1hT_t[:, kh, :] for kh in range(2)],
                    [r1cT_t[:, kh, :] for kh in range(2)], r1b_b, "r1")
    x2, x2_16 = [], []
    for chh in range(2):
        nc.gpsimd.dma_start(d["r1hT_o"][ts(chh, 128), :], r1h[chh][:])
        nc.gpsimd.dma_start(d["r1cT_o"][ts(chh, 128), :], r1c[chh][:])
        xx = wp.tile([128, BC], F32, tag=f"x2{chh}")
        nc.vector.tensor_add(xx[:], xT[chh][:], r1h[chh][:])
        x2.append(xx)
        xx16 = wp.tile([128, BC], F16, tag=f"x216{chh}")
        nc.vector.tensor_copy(xx16[:], xx[:])
        x2_16.append(xx16)

    r2h, r2c = lstm([x2_16[0][:], x2_16[1][:]], r2wih_t, r2whh_t,
                    [r2hT_t[:, kh, :] for kh in range(2)],
                    [r2cT_t[:, kh, :] for kh in range(2)], r2b_b, "r2")
    x3 = []
    for chh in range(2):
        nc.gpsimd.dma_start(d["r2hT_o"][ts(chh, 128), :], r2h[chh][:])
        nc.gpsimd.dma_start(d["r2cT_o"][ts(chh, 128), :], r2c[chh][:])
        xx = wp.tile([128, BC], F16, tag=f"x3{chh}")
        nc.vector.tensor_add(xx[:], x2[chh][:], r2h[chh][:])
        x3.append(xx)

    ps3 = dps.tile([128, BC], F32, tag="dps")
    for ki in range(2):
        nc.tensor.matmul(ps3[:], projw_t[:, ki, 0:128], x3[ki][:],
                         start=(ki == 0), stop=(ki == 1))
    o0 = tp.tile([128, BC], F32, tag="o0")
    nc.vector.tensor_copy(o0[:], ps3[:])
    nc.gpsimd.dma_start(d["outT_o"][0:128, :], o0[:])
    ps4 = dps.tile([32, BC], F32, tag="dps")
    for ki in range(2):
        nc.tensor.matmul(ps4[:], projw_t[:, ki, 128:160], x3[ki][:],
                         start=(ki == 0), stop=(ki == 1))
    o1 = tp.tile([32, BC], F32, tag="o1")
    nc.vector.tensor_copy(o1[:], ps4[:])
    nc.gpsimd.dma_start(d["outT_o"][128:160, :], o1[:])


def _host_prepare(inputs):
    """Build the 8 per-core input maps from the full inputs."""
    f = np.float32
    f16 = np.float16
    npa = {k: np.asarray(v, dtype=f) for k, v in inputs.items()}

    def TT(a, dt=f):
        return np.ascontiguousarray(a.T.astype(dt))

    def b2(bias, n):
        return np.ascontiguousarray(np.asarray(bias, f).reshape(n, 128).T)

    wk = {
        "fc1_wT": TT(npa["fc1_w"], f16),
        "fc1_b2": b2(npa["fc1_b"], 2),
        "fc2_wT": TT(npa["fc2_w"], f16),
        "fc2_b2": b2(npa["fc2_b"], 2),
        "attn_wihT": TT(npa["attn_wih"], f16),
        "attn_whhT": TT(npa["attn_whh"], f16),
        "attn_b2": b2(npa["attn_bih"] + npa["attn_bhh"], 8),
        "W_wT": TT(npa["W_w"], f16),
        "W_b2": b2(npa["W_b"], 1),
        "V_wT": TT(npa["V_w"], f16),
        "T_b2": b2(npa["T_b"], 1),
        "lin_wT": TT(npa["lin_w"], f16),
        "lin_b2": b2(npa["lin_b"], 2),
        "r1_wihT": TT(npa["r1_wih"], f16),
        "r1_whhT": TT(npa["r1_whh"], f16),
        "r1_b2": b2(npa["r1_bih"] + npa["r1_bhh"], 8),
        "r2_wihT": TT(npa["r2_wih"], f16),
        "r2_whhT": TT(npa["r2_whh"], f16),
        "r2_b2": b2(npa["r2_bih"] + npa["r2_bhh"], 8),
        "proj_wT": TT(npa["proj_w"], f16),
    }
    wk["TwT"] = np.ascontiguousarray(npa["T_w"].T.astype(f16))
    wk["CUstatic"] = np.ascontiguousarray(
        npa["F_w"][:, 0, :].T @ npa["U_w"].T)
    vbig = np.zeros((128, 2 * NG - 1), f16)
    vbig[:, NG - 1] = npa["v_w"][0].astype(f16)
    wk["v_big"] = vbig

    pbc = np.ascontiguousarray(
        np.broadcast_to(npa["P"][None, :], (BC, PL)).astype(f))

    in_maps = []
    for c in range(NCORES):
        sl = slice(c * BC, (c + 1) * BC)
        m = dict(wk)
        m["h"] = np.ascontiguousarray(npa["h"][sl]).astype(f16)
        m["yT"] = TT(npa["y"][sl], f16)
        m["cT"] = TT(npa["c"][sl], f16)
        m["ahT"] = TT(npa["attn_h0"][sl], f16)
        m["acT"] = TT(npa["attn_c0"][sl])
        m["r1hT"] = TT(npa["rnn1_h0"][sl], f16)
        m["r1cT"] = TT(npa["rnn1_c0"][sl])
        m["r2hT"] = TT(npa["rnn2_h0"][sl], f16)
        m["r2cT"] = TT(npa["rnn2_c0"][sl])
        halo = np.ascontiguousarray(
            np.pad(npa["alpha"][sl], ((0, 0), (K21 // 2, K21 // 2))))
        m["alpha_halo"] = halo
        m["alpha_halo16"] = halo.astype(f16)
        m["P_bc"] = pbc
        in_maps.append(m)
    return in_maps


def _assemble(results):
    outs, a2s, c2s = [], [], []
    ahs, acs, r1hs, r1cs, r2hs, r2cs = [], [], [], [], [], []
    for r in results:
        outs.append(r["outT"].T.reshape(BC, N_MELS, R))
        a2s.append(r["alpha2"])
        c2s.append(r["c2"])
        ahs.append(r["ahT_o"].T)
        acs.append(r["acT_o"].T)
        r1hs.append(r["r1hT_o"].T)
        r1cs.append(r["r1cT_o"].T)
        r2hs.append(r["r2hT_o"].T)
        r2cs.append(r["r2cT_o"].T)

    def cat(xs):
        return np.ascontiguousarray(np.concatenate(xs, axis=0))

    return (cat(outs), cat(a2s), cat(c2s), cat(ahs), cat(acs),
            cat(r1hs), cat(r1cs), cat(r2hs), cat(r2cs))


def get_program():
    global _PROG
    if _PROG is None:
        _PROG = _build_program()
    return _PROG


def kernel(**inputs):
    nc = get_program()
    in_maps = _host_prepare(inputs)
    res = run_bass_kernel_spmd(nc, in_maps, list(range(NCORES)))
    return _assemble(res.results)


if __name__ == "__main__":
    get_program()
    print("program built OK")
